# revision 12
# baseline (speedup 1.0000x reference)
"""Trainium2 Bass kernel for nn_CAModel (neural cellular automaton step).

v2 — restructured from the 304us baseline around three trace findings:
(1) TensorE was 72% busy on 528 serialized matmul+ldweights pairs,
(2) the PSUM->SBUF relu drain (8.4M elem/core) must be split DVE/ACT,
(3) odd-column-offset conv ops fall off the DVE 2x fast path.

Layout (per core, 4 images): partitions p = (img 4, half 2, chan 16),
free dim = padded half-image rows x 132 pitch (keeps 4B alignment).

- conv: shifted-output formulation so every tensor_tensor op has even
  element offsets (DVE 2x); the x2 scale rides ScalarE's activation
  scale; banded temporaries, ops interleaved between chunk drains.
- fc0: weight-major phases per (feat, half-parity); one replicated
  [128,128] weight serves 4 concurrent row-tiled K=32 matmuls.
- PSUM: two [128,2048] 4-bank tiles ping-pong across half-parities; fc1
  dx accumulates into bank 0 of the first-drained tile (8 banks exact).
- relu drain split ScalarE[0:DA] / VectorE[DA:2048].
- residual + update mask per chunk; alive masks in strip layout; life
  broadcast to channels via PE selector matmuls; bf16 output.
"""

import dataclasses
import numpy as np
import ml_dtypes

import concourse.bass as bass
import concourse.tile as tile
from concourse import mybir, bass_utils

F32 = mybir.dt.float32
BF16 = mybir.dt.bfloat16
ALU = mybir.AluOpType
ACTF = mybir.ActivationFunctionType

N_CORES = 8
B, H, W, C = 32, 128, 128, 16
HID = 128
IMGS = B // N_CORES          # 4 images per core
PW = 132                     # padded row pitch (4B-aligned shifts)
PR = 66                      # padded rows per half (1 + 64 + 1)
NPAD = PR * PW               # 8712
NPIX = 64 * W                # 8192 interior pixels per (img,half)
G = 128                      # guard elems around x2 free dim
PITCH = NPIX + 2 * G         # x2 tile span
NCHUNK = 16                  # chunks of 4 interior rows
CN = 512                     # pixels per (img,half) per chunk

# Tunables
DA = 1536                    # relu drain split: ACT [0:DA], DVE [DA:2048]
YDY_ON_GPSIMD = False         # ydys pass on GpSimd (else DVE)
X2_ON_GPSIMD = False          # residual add on GpSimd (else DVE)


def _split_multiwaits(nc):
    """walrus in this env only supports one sem-wait per instruction."""
    n = 0
    for f in nc.m.functions:
        for bb in f.blocks:
            out = []
            changed = False
            for inst in bb.instructions:
                si = inst.sync_info
                if si is not None and len(si.on_wait) > 1:
                    waits = list(si.on_wait)
                    for k, w in enumerate(waits[:-1]):
                        nop = mybir.InstNoOp(
                            name=f"{inst.name}_ws{k}",
                            sync_info=mybir.SyncInfo(on_wait=[w], on_update=[]),
                            bass_nofuse=True,
                            engine=inst.engine,
                        )
                        nc.register_instruction(nop, overwrite=True)
                        out.append(nop)
                        n += 1
                    inst.sync_info = mybir.SyncInfo(
                        on_wait=[waits[-1]], on_update=list(si.on_update)
                    )
                    changed = True
                out.append(inst)
            if changed:
                bb.instructions[:] = out
    return n


def _mk_ap(ap, offset, dims):
    return dataclasses.replace(ap, offset=offset, ap=[list(d) for d in dims])


def build_program():
    nc = bass.Bass()

    xpad_d = nc.dram_tensor("xpad", [128, NPAD], BF16, kind="ExternalInput").ap()
    u16_d = nc.dram_tensor("u16", [128, NPIX], BF16, kind="ExternalInput").ap()
    astrip_d = nc.dram_tensor("astrip", [128, 780], F32, kind="ExternalInput").ap()
    w0_d = {}
    for feat in ("id", "dx", "dy"):
        for gg in range(2):
            w0_d[(feat, gg)] = nc.dram_tensor(
                f"w0{feat}{gg}", [128, 128], BF16, kind="ExternalInput"
            ).ap()
    w1_d = [
        nc.dram_tensor(f"w1{gg}", [128, 32], BF16, kind="ExternalInput").ap()
        for gg in range(2)
    ]
    sel_d = nc.dram_tensor("sel", [128, 2048], BF16, kind="ExternalInput").ap()
    out_d = nc.dram_tensor("out", [128, NPIX], BF16, kind="ExternalOutput").ap()

    XBANDS = [(0, 8), (8, 19), (19, 30), (30, 41), (41, 52), (52, 66)]

    with tile.TileContext(nc) as tc:
        with (
            tc.tile_pool(name="persist", bufs=1) as pp,
            tc.tile_pool(name="psum", bufs=1, space="PSUM") as psp,
        ):
            xpad = pp.tile([128, NPAD + 4], BF16, tag="xpad")
            u16 = pp.tile([128, NPIX], BF16, tag="u16")
            x2 = pp.tile([128, PITCH], BF16, tag="x2")
            astrip = pp.tile([128, 780], F32, tag="astrip")
            a2strip = pp.tile([128, 780], BF16, tag="a2strip")
            prepool = pp.tile([128, 512], F32, tag="prepool")
            selt = pp.tile([128, 2048], BF16, tag="selt")
            w0t = {
                k: pp.tile([128, 128], BF16, tag=f"w0{k[0]}{k[1]}",
                           name=f"w0t{k[0]}{k[1]}")
                for k in w0_d
            }
            w1t = [
                pp.tile([128, 32], BF16, tag=f"w1{gg}", name=f"w1t{gg}")
                for gg in range(2)
            ]

            # ---- input DMAs (xpad first; bulk on the cheap Pool queue) ----
            for lo, hi in XBANDS:
                nc.sync.dma_start(
                    out=xpad[:, lo * PW : hi * PW], in_=xpad_d[:, lo * PW : hi * PW]
                )
            for k in w0_d:
                nc.gpsimd.dma_start(out=w0t[k][:, :], in_=w0_d[k])
            for gg in range(2):
                nc.gpsimd.dma_start(out=w1t[gg][:, :], in_=w1_d[gg])
            nc.gpsimd.dma_start(out=astrip[:, :], in_=astrip_d)
            nc.gpsimd.dma_start(out=selt[:, :], in_=sel_d)
            for ub in range(4):
                nc.gpsimd.dma_start(
                    out=u16[:, ub * 2048 : (ub + 1) * 2048],
                    in_=u16_d[:, ub * 2048 : (ub + 1) * 2048],
                )

            nc.gpsimd.memset(x2[:, 0:G], 0.0)
            nc.gpsimd.memset(x2[:, G + NPIX : PITCH], 0.0)
            nc.gpsimd.memset(a2strip[:, :], 0.0)

            # ---- conv (shifted-output, all-even offsets) ----
            # th_b[rr,c] = 2*x[pa,c+1] (ACT) then += s_b  == th(pa, c+1)
            # s_b[rr,c]  = x[pa,c] + x[pa,c+2]
            # v_b[rr,c]  = x[pa,c] + x[pa+1,c]
            # tv_b[rr,c] = v[rr,c] + v[rr+1,c]      (true position)
            # yx_b[rr,c] = tv[rr,c+2] - tv[rr,c]    == ydx(., c+1)
            # yy_b[rr,c] = th[rr+2,c] - th[rr,c]    == ydy(., c+1)
            cp = tc.tile_pool(name="conv", bufs=1)
            cpx = cp.__enter__()
            band_tiles = {}

            def alloc_band(b):
                band_tiles[b] = (
                    cpx.tile([128, 18 * PW], BF16, tag="cs", bufs=2,
                             name=f"cs{b}"),
                    cpx.tile([128, 18 * PW], BF16, tag="cth", bufs=2,
                             name=f"cth{b}"),
                    cpx.tile([128, 17 * PW], BF16, tag="cv", bufs=2,
                             name=f"cv{b}"),
                    cpx.tile([128, 16 * PW + 4], BF16, tag="ctv", bufs=2,
                             name=f"ctv{b}"),
                    cpx.tile([128, 16 * PW], BF16, tag="cyx", bufs=2,
                             name=f"cyx{b}"),
                    cpx.tile([128, 16 * PW], BF16, tag="cyy", bufs=2,
                             name=f"cyy{b}"),
                )

            def conv_ops(b, lo, hi):
                """Thunks for interior rows [16b+lo, 16b+hi)."""
                s_b, th_b, v_b, tv_b, yx_b, yy_b = band_tiles[b]
                base = 16 * b
                ydy_eng = nc.gpsimd if YDY_ON_GPSIMD else nc.vector
                return [
                    lambda: nc.scalar.activation(
                        out=th_b[:, lo * PW : (hi + 2) * PW],
                        in_=xpad[:, (base + lo) * PW + 1 : (base + hi + 2) * PW + 1],
                        func=ACTF.Copy, scale=2.0,
                    ),
                    lambda: nc.vector.tensor_tensor(
                        out=s_b[:, lo * PW : (hi + 2) * PW],
                        in0=xpad[:, (base + lo) * PW : (base + hi + 2) * PW],
                        in1=xpad[:, (base + lo) * PW + 2 : (base + hi + 2) * PW + 2],
                        op=ALU.add,
                    ),
                    lambda: nc.vector.tensor_tensor(
                        out=th_b[:, lo * PW : (hi + 2) * PW],
                        in0=th_b[:, lo * PW : (hi + 2) * PW],
                        in1=s_b[:, lo * PW : (hi + 2) * PW],
                        op=ALU.add,
                    ),
                    lambda: nc.vector.tensor_tensor(
                        out=v_b[:, lo * PW : (hi + 1) * PW],
                        in0=xpad[:, (base + lo) * PW : (base + hi + 1) * PW],
                        in1=xpad[:, (base + lo + 1) * PW : (base + hi + 2) * PW],
                        op=ALU.add,
                    ),
                    lambda: nc.vector.tensor_tensor(
                        out=tv_b[:, lo * PW : hi * PW],
                        in0=v_b[:, lo * PW : hi * PW],
                        in1=v_b[:, (lo + 1) * PW : (hi + 1) * PW],
                        op=ALU.add,
                    ),
                    lambda: nc.vector.tensor_tensor(
                        out=yx_b[:, lo * PW : hi * PW],
                        in0=tv_b[:, lo * PW + 2 : hi * PW + 2],
                        in1=tv_b[:, lo * PW : hi * PW],
                        op=ALU.subtract,
                    ),
                    lambda: ydy_eng.tensor_tensor(
                        out=yy_b[:, lo * PW : hi * PW],
                        in0=th_b[:, (lo + 2) * PW : (hi + 2) * PW],
                        in1=th_b[:, lo * PW : hi * PW],
                        op=ALU.subtract,
                    ),
                ]

            # prologue: band 0 in 4-row sub-bands so chunk-0 matmuls start early
            alloc_band(0)
            for sb in range(4):
                for op in conv_ops(0, 4 * sb, 4 * sb + 4):
                    op()
            # band b+1's ops are emitted only during band b's chunks —
            # emitting band b+2 early would head-block the in-order DVE
            # stream on a WAR dep (its buffers are still being read).
            pend = {}
            for b in range(1, 4):
                alloc_band(b)
                pend[b] = conv_ops(b, 0, 16)

            def emit_prepool():
                t1 = pp.tile([128, 524], F32, tag="pp_t1")
                vm = pp.tile([128, 524], F32, tag="pp_vm")
                t2 = pp.tile([128, 524], F32, tag="pp_t2")
                nc.vector.tensor_tensor(
                    out=t1[:, 0:520], in0=astrip[:, 0:520],
                    in1=astrip[:, 130:650], op=ALU.max,
                )
                nc.vector.tensor_tensor(
                    out=vm[:, 0:520], in0=t1[:, 0:520],
                    in1=astrip[:, 260:780], op=ALU.max,
                )
                nc.vector.tensor_tensor(
                    out=t2[:, 0:519], in0=vm[:, 0:519], in1=vm[:, 1:520],
                    op=ALU.max,
                )
                vmr = vm[:, 0:520].rearrange("p (r w) -> p r w", w=130)
                t2r = t2[:, 0:520].rearrange("p (r w) -> p r w", w=130)
                ppr = prepool[:, :].rearrange("p (r w) -> p r w", w=128)
                nc.vector.tensor_tensor(
                    out=ppr[:, 0:4, :], in0=t2r[:, 0:4, 0:128],
                    in1=vmr[:, 0:4, 2:130], op=ALU.max,
                )

            # ---- main chunk loop ----
            A = psp.tile([128, 2048], F32, tag="psA")
            Bp = psp.tile([128, 2048], F32, tag="psB")
            hp = [A, Bp]
            xpr = xpad[:, 0:NPAD].rearrange("p (r w) -> p r w", w=PW)

            for k in range(NCHUNK):
                if k == 2:
                    emit_prepool()
                b = k // 4
                lr0 = 4 * k - 16 * b
                yx_r = band_tiles[b][4][:, :].rearrange("p (r w) -> p r w", w=PW)
                yy_r = band_tiles[b][5][:, :].rearrange("p (r w) -> p r w", w=PW)
                rhss = [
                    xpr[:, 1 + 4 * k : 5 + 4 * k, 2:130],
                    yx_r[:, lr0 : lr0 + 4, 1:129],
                    yy_r[:, lr0 : lr0 + 4, 1:129],
                ]
                Q = hp[k % 2]           # dxp accumulator (bank 0)
                # PE stream stays gapless: both fc0 phases back-to-back,
                # then fc1 (drain deps resolve while fc0-gg1 runs)
                for gg in range(2):
                    P = hp[(k + gg) % 2]
                    for fi, feat in enumerate(("id", "dx", "dy")):
                        for j in range(4):
                            nc.tensor.matmul(
                                P[:, 512 * j : 512 * j + 512],
                                w0t[(feat, gg)][32 * j : 32 * j + 32, :],
                                rhss[fi][32 * j : 32 * j + 32],
                                start=(fi == 0),
                                stop=(fi == 2),
                                tile_position=(32 * j, 0),
                            )
                rhs_t = []
                for gg in range(2):
                    P = hp[(k + gg) % 2]
                    rh = pp.tile([128, 2048], BF16, tag=f"rh{gg}", bufs=2,
                                 name=f"rh{gg}_{k}")
                    # per-512 drain slices so each fc1 matmul starts as soon
                    # as its slice lands; 3 on ScalarE, 1 on VectorE
                    for j in range(3):
                        nc.scalar.activation(
                            out=rh[:, 512 * j : 512 * j + 512],
                            in_=P[:, 512 * j : 512 * j + 512],
                            func=ACTF.Relu,
                        )
                    nc.vector.tensor_scalar_max(
                        rh[:, 1536:2048], P[:, 1536:2048], 0.0
                    )
                    rhs_t.append(rh)
                for gg in range(2):
                    for j in range(4):
                        nc.tensor.matmul(
                            Q[32 * j : 32 * j + 32, 0:512],
                            w1t[gg][:, :],
                            rhs_t[gg][:, 512 * j : 512 * j + 512],
                            start=(gg == 0),
                            stop=(gg == 1),
                            tile_position=(0, 32 * j),
                        )
                nxt = pend.get(b + 1)
                for _ in range(4):
                    if nxt:
                        nxt.pop(0)()
                st = pp.tile([128, 512], BF16, tag="st", bufs=2)
                nc.vector.tensor_tensor(
                    out=st[:, :],
                    in0=Q[:, 0:512],
                    in1=u16[:, k * CN : (k + 1) * CN],
                    op=ALU.mult,
                )
                x2_eng = nc.gpsimd if X2_ON_GPSIMD else nc.vector
                x2_eng.tensor_tensor(
                    out=x2[:, G + k * CN : G + (k + 1) * CN],
                    in0=xpr[:, 1 + 4 * k : 5 + 4 * k, 2:130],
                    in1=st[:, :],
                    op=ALU.add,
                )

            cpx_exc = cp.__exit__(None, None, None)
            del cpx_exc

            # ---- alive masks ----
            x2ap = x2[:, :]
            a2ap = a2strip[:, :]
            for r in range(6):
                src = _mk_ap(
                    x2ap, 3 * PITCH + 128 * r,
                    [[16 * PITCH, 8], [512, 16], [1, 128]],
                )
                dst = _mk_ap(a2ap, 130 * r + 1, [[780, 128], [1, 128]])
                nc.gpsimd.dma_start(out=dst, in_=src)
            nc.gpsimd.dma_start(
                out=_mk_ap(a2ap, 15 * 780 + 5 * 130 + 1, [[32 * 780, 4], [1, 128]]),
                in_=_mk_ap(x2ap, 19 * PITCH + G, [[32 * PITCH, 4], [1, 128]]),
            )
            nc.gpsimd.dma_start(
                out=_mk_ap(a2ap, 16 * 780 + 1, [[32 * 780, 4], [1, 128]]),
                in_=_mk_ap(
                    x2ap, 3 * PITCH + G + 63 * 128, [[32 * PITCH, 4], [1, 128]]
                ),
            )

            postpool = pp.tile([128, 512], BF16, tag="postpool")
            t1b = pp.tile([128, 524], BF16, tag="ap_t1")
            vmb = pp.tile([128, 524], BF16, tag="ap_vm")
            t2b = pp.tile([128, 524], BF16, tag="ap_t2")
            nc.vector.tensor_tensor(
                out=t1b[:, 0:520], in0=a2strip[:, 0:520],
                in1=a2strip[:, 130:650], op=ALU.max,
            )
            nc.vector.tensor_tensor(
                out=vmb[:, 0:520], in0=t1b[:, 0:520],
                in1=a2strip[:, 260:780], op=ALU.max,
            )
            nc.vector.tensor_tensor(
                out=t2b[:, 0:519], in0=vmb[:, 0:519], in1=vmb[:, 1:520],
                op=ALU.max,
            )
            vmr2 = vmb[:, 0:520].rearrange("p (r w) -> p r w", w=130)
            t2r2 = t2b[:, 0:520].rearrange("p (r w) -> p r w", w=130)
            ppr2 = postpool[:, :].rearrange("p (r w) -> p r w", w=128)
            nc.vector.tensor_tensor(
                out=ppr2[:, 0:4, :], in0=t2r2[:, 0:4, 0:128],
                in1=vmr2[:, 0:4, 2:130], op=ALU.max,
            )
            pmin = pp.tile([128, 512], BF16, tag="pmin")
            nc.vector.tensor_tensor(
                out=pmin[:, :], in0=prepool[:, :], in1=postpool[:, :], op=ALU.min
            )
            lifes = pp.tile([128, 512], BF16, tag="lifes")
            nc.vector.tensor_scalar(
                out=lifes[:, :], in0=pmin[:, :], scalar1=0.1, scalar2=None,
                op0=ALU.is_gt,
            )

            # ---- life broadcast (PE) + final mask multiply + store ----
            for k4 in range(4):
                lps = hp[k4 % 2]
                for tl in range(4):
                    t = 4 * k4 + tl
                    nc.tensor.matmul(
                        lps[:, 512 * tl : 512 * tl + 512],
                        selt[:, 128 * t : 128 * t + 128],
                        lifes[:, 0:512],
                        start=True,
                        stop=True,
                    )
                ot = pp.tile([128, 2048], BF16, tag="ot", bufs=2)
                nc.vector.tensor_tensor(
                    out=ot[:, :],
                    in0=x2[:, G + 2048 * k4 : G + 2048 * (k4 + 1)],
                    in1=lps[:, :],
                    op=ALU.mult,
                )
                nc.sync.dma_start(
                    out=out_d[:, 2048 * k4 : 2048 * (k4 + 1)], in_=ot[:, :]
                )

    _split_multiwaits(nc)
    return nc


def host_prep(x, w0, w1, rand_mask):
    bf = ml_dtypes.bfloat16
    xt = np.ascontiguousarray(x.transpose(0, 3, 1, 2))  # [B, C, H, W]

    xp = np.zeros((B, 2, C, PR, PW), np.float32)
    xp[:, 0, :, 1:66, 2:130] = xt[:, :, 0:65, :]
    xp[:, 1, :, 0:65, 2:130] = xt[:, :, 63:128, :]
    xp = xp.astype(bf).reshape(B, 2, C, NPAD)

    u = (rand_mask[..., 0] <= 0.5).astype(np.float32).reshape(B, 2, 64, W)
    u16 = np.ascontiguousarray(
        np.broadcast_to(u[:, :, None], (B, 2, C, 64, W))
    ).astype(bf).reshape(B, 2, C, NPIX)

    apad = np.zeros((B, H + 2, 130), np.float32)
    apad[:, 1:129, 1:129] = x[..., 3]
    idx = 4 * np.arange(32)[:, None] + np.arange(6)[None, :]
    astr = apad[:, idx, :].reshape(B, 32, 780)  # [B, strip, 6*130]

    W0id = w0[:, 0::3]
    W0dx = w0[:, 1::3] / 8.0
    W0dy = w0[:, 2::3] / 8.0
    w0_arrs = {}
    for feat, Wm in (("id", W0id), ("dx", W0dx), ("dy", W0dy)):
        blk = Wm.T.astype(bf)  # [16 c, 128 o]
        for gg in range(2):
            t = np.zeros((128, 128), bf)
            for j in range(4):
                t[32 * j + 16 * gg : 32 * j + 16 * gg + 16, :] = blk
            w0_arrs[(feat, gg)] = t
    w1_arrs = []
    for gg in range(2):
        t = np.zeros((128, 32), bf)
        t[:, 16 * gg : 16 * gg + 16] = w1.T.astype(bf)
        w1_arrs.append(t)

    sel = np.zeros((128, 2048), bf)
    for t in range(16):
        for p in range(128):
            g = p // 16
            sel[16 * g + t, 128 * t + p] = 1.0

    in_maps = []
    for ci in range(N_CORES):
        sl = slice(IMGS * ci, IMGS * (ci + 1))
        m = {
            "xpad": np.ascontiguousarray(xp[sl]).reshape(128, NPAD),
            "u16": np.ascontiguousarray(u16[sl]).reshape(128, NPIX),
            "astrip": np.ascontiguousarray(astr[sl]).reshape(128, 780),
            "sel": sel,
            "w10": w1_arrs[0],
            "w11": w1_arrs[1],
        }
        for (feat, gg), arr in w0_arrs.items():
            m[f"w0{feat}{gg}"] = arr
        in_maps.append(m)
    return in_maps


def host_post(results):
    out = np.empty((B, H, W, C), np.float32)
    for ci in range(N_CORES):
        o = results[ci]["out"].astype(np.float32).reshape(IMGS, 2, C, 64, W)
        out[IMGS * ci : IMGS * (ci + 1)] = o.transpose(0, 1, 3, 4, 2).reshape(
            IMGS, H, W, C
        )
    return out


_CACHE = {}


def kernel(x, w0, w1, rand_mask, _trace=False):
    x = np.asarray(x, np.float32)
    w0 = np.asarray(w0, np.float32)
    w1 = np.asarray(w1, np.float32)
    rand_mask = np.asarray(rand_mask, np.float32)

    if "nc" not in _CACHE:
        _CACHE["nc"] = build_program()
    nc = _CACHE["nc"]

    in_maps = host_prep(x, w0, w1, rand_mask)
    res = bass_utils.run_bass_kernel_spmd(
        nc, in_maps, core_ids=list(range(N_CORES)), trace=_trace
    )
    _CACHE["last_result"] = res
    return host_post(res.results)


# revision 14
# speedup vs baseline: 1.1373x; 1.1373x over previous
"""Trainium2 Bass kernel for nn_CAModel (neural cellular automaton step).

v2 — restructured from the 304us baseline around three trace findings:
(1) TensorE was 72% busy on 528 serialized matmul+ldweights pairs,
(2) the PSUM->SBUF relu drain (8.4M elem/core) must be split DVE/ACT,
(3) odd-column-offset conv ops fall off the DVE 2x fast path.

Layout (per core, 4 images): partitions p = (img 4, half 2, chan 16),
free dim = padded half-image rows x 132 pitch (keeps 4B alignment).

- conv: shifted-output formulation so every tensor_tensor op has even
  element offsets (DVE 2x); the x2 scale rides ScalarE's activation
  scale; banded temporaries, ops interleaved between chunk drains.
- fc0: weight-major phases per (feat, half-parity); one replicated
  [128,128] weight serves 4 concurrent row-tiled K=32 matmuls.
- PSUM: two [128,2048] 4-bank tiles ping-pong across half-parities; fc1
  dx accumulates into bank 0 of the first-drained tile (8 banks exact).
- relu drain split ScalarE[0:DA] / VectorE[DA:2048].
- residual + update mask per chunk; alive masks in strip layout; life
  broadcast to channels via PE selector matmuls; bf16 output.
"""

import dataclasses
import numpy as np
import ml_dtypes

import concourse.bass as bass
import concourse.tile as tile
from concourse import mybir, bass_utils

F32 = mybir.dt.float32
BF16 = mybir.dt.bfloat16
ALU = mybir.AluOpType
ACTF = mybir.ActivationFunctionType

N_CORES = 8
B, H, W, C = 32, 128, 128, 16
HID = 128
IMGS = B // N_CORES          # 4 images per core
PW = 132                     # padded row pitch (4B-aligned shifts)
PR = 66                      # padded rows per half (1 + 64 + 1)
NPAD = PR * PW               # 8712
NPIX = 64 * W                # 8192 interior pixels per (img,half)
NCHUNK = 16                  # chunks of 4 interior rows
CN = 512                     # pixels per (img,half) per chunk

# Tunables
DA = 1536                    # relu drain split: ACT [0:DA], DVE [DA:2048]
YDY_ON_GPSIMD = False         # ydys pass on GpSimd (else DVE)
X2_ON_GPSIMD = False          # residual add on GpSimd (else DVE)


def _split_multiwaits(nc):
    """walrus in this env only supports one sem-wait per instruction."""
    n = 0
    for f in nc.m.functions:
        for bb in f.blocks:
            out = []
            changed = False
            for inst in bb.instructions:
                si = inst.sync_info
                if si is not None and len(si.on_wait) > 1:
                    waits = list(si.on_wait)
                    for k, w in enumerate(waits[:-1]):
                        nop = mybir.InstNoOp(
                            name=f"{inst.name}_ws{k}",
                            sync_info=mybir.SyncInfo(on_wait=[w], on_update=[]),
                            bass_nofuse=True,
                            engine=inst.engine,
                        )
                        nc.register_instruction(nop, overwrite=True)
                        out.append(nop)
                        n += 1
                    inst.sync_info = mybir.SyncInfo(
                        on_wait=[waits[-1]], on_update=list(si.on_update)
                    )
                    changed = True
                out.append(inst)
            if changed:
                bb.instructions[:] = out
    return n


def _mk_ap(ap, offset, dims):
    return dataclasses.replace(ap, offset=offset, ap=[list(d) for d in dims])


def build_program():
    nc = bass.Bass()

    xpad_d = nc.dram_tensor("xpad", [128, NPAD], BF16, kind="ExternalInput").ap()
    u16_d = nc.dram_tensor("u16", [128, NPIX], BF16, kind="ExternalInput").ap()
    astrip_d = nc.dram_tensor("astrip", [128, 780], F32, kind="ExternalInput").ap()
    w0_d = {}
    for feat in ("id", "dx", "dy"):
        for gg in range(2):
            w0_d[(feat, gg)] = nc.dram_tensor(
                f"w0{feat}{gg}", [128, 128], BF16, kind="ExternalInput"
            ).ap()
    w1_d = [
        nc.dram_tensor(f"w1{gg}", [128, 32], BF16, kind="ExternalInput").ap()
        for gg in range(2)
    ]
    sel_d = nc.dram_tensor("sel", [128, 2048], BF16, kind="ExternalInput").ap()
    out_d = nc.dram_tensor("out", [128, NPIX], BF16, kind="ExternalOutput").ap()

    XBANDS = [(0, 8), (8, 19), (19, 30), (30, 41), (41, 52), (52, 66)]

    with tile.TileContext(nc) as tc:
        with (
            tc.tile_pool(name="persist", bufs=1) as pp,
            tc.tile_pool(name="psum", bufs=1, space="PSUM") as psp,
        ):
            xpad = pp.tile([128, NPAD + 4], BF16, tag="xpad")
            u16 = pp.tile([128, NPIX], BF16, tag="u16")
            x2 = pp.tile([128, NPAD], BF16, tag="x2")
            astrip = pp.tile([128, 780], F32, tag="astrip")
            a2strip = pp.tile([128, 780], BF16, tag="a2strip")
            prepool = pp.tile([128, 512], F32, tag="prepool")
            selt = pp.tile([128, 2048], BF16, tag="selt")
            w0t = {
                k: pp.tile([128, 128], BF16, tag=f"w0{k[0]}{k[1]}",
                           name=f"w0t{k[0]}{k[1]}")
                for k in w0_d
            }
            w1t = [
                pp.tile([128, 32], BF16, tag=f"w1{gg}", name=f"w1t{gg}")
                for gg in range(2)
            ]

            # ---- input DMAs (xpad first; bulk on the cheap Pool queue) ----
            for lo, hi in XBANDS:
                nc.sync.dma_start(
                    out=xpad[:, lo * PW : hi * PW], in_=xpad_d[:, lo * PW : hi * PW]
                )
            for k in w0_d:
                nc.gpsimd.dma_start(out=w0t[k][:, :], in_=w0_d[k])
            for gg in range(2):
                nc.gpsimd.dma_start(out=w1t[gg][:, :], in_=w1_d[gg])
            nc.gpsimd.dma_start(out=astrip[:, :], in_=astrip_d)
            nc.gpsimd.dma_start(out=selt[:, :], in_=sel_d)
            for ub in range(4):
                nc.gpsimd.dma_start(
                    out=u16[:, ub * 2048 : (ub + 1) * 2048],
                    in_=u16_d[:, ub * 2048 : (ub + 1) * 2048],
                )

            nc.gpsimd.memset(x2[:, 0:PW], 0.0)
            nc.gpsimd.memset(x2[:, 65 * PW : NPAD], 0.0)
            nc.gpsimd.memset(a2strip[:, :], 0.0)

            # ---- conv (shifted-output, all-even offsets) ----
            # th_b[rr,c] = 2*x[pa,c+1] (ACT) then += s_b  == th(pa, c+1)
            # s_b[rr,c]  = x[pa,c] + x[pa,c+2]
            # v_b[rr,c]  = x[pa,c] + x[pa+1,c]
            # tv_b[rr,c] = v[rr,c] + v[rr+1,c]      (true position)
            # yx_b[rr,c] = tv[rr,c+2] - tv[rr,c]    == ydx(., c+1)
            # yy_b[rr,c] = th[rr+2,c] - th[rr,c]    == ydy(., c+1)
            cp = tc.tile_pool(name="conv", bufs=1)
            cpx = cp.__enter__()
            band_tiles = {}

            def alloc_band(b):
                band_tiles[b] = (
                    cpx.tile([128, 18 * PW], BF16, tag="cs", bufs=2,
                             name=f"cs{b}"),
                    cpx.tile([128, 18 * PW], BF16, tag="cth", bufs=2,
                             name=f"cth{b}"),
                    cpx.tile([128, 17 * PW], BF16, tag="cv", bufs=2,
                             name=f"cv{b}"),
                    cpx.tile([128, 16 * PW + 4], BF16, tag="ctv", bufs=2,
                             name=f"ctv{b}"),
                    cpx.tile([128, 16 * PW], BF16, tag="cyx", bufs=2,
                             name=f"cyx{b}"),
                    cpx.tile([128, 16 * PW], BF16, tag="cyy", bufs=2,
                             name=f"cyy{b}"),
                )

            def conv_ops(b, lo, hi):
                """Thunks for interior rows [16b+lo, 16b+hi)."""
                s_b, th_b, v_b, tv_b, yx_b, yy_b = band_tiles[b]
                base = 16 * b
                ydy_eng = nc.gpsimd if YDY_ON_GPSIMD else nc.vector
                return [
                    lambda: nc.scalar.activation(
                        out=th_b[:, lo * PW : (hi + 2) * PW],
                        in_=xpad[:, (base + lo) * PW + 1 : (base + hi + 2) * PW + 1],
                        func=ACTF.Copy, scale=2.0,
                    ),
                    lambda: nc.vector.tensor_tensor(
                        out=s_b[:, lo * PW : (hi + 2) * PW],
                        in0=xpad[:, (base + lo) * PW : (base + hi + 2) * PW],
                        in1=xpad[:, (base + lo) * PW + 2 : (base + hi + 2) * PW + 2],
                        op=ALU.add,
                    ),
                    lambda: nc.vector.tensor_tensor(
                        out=th_b[:, lo * PW : (hi + 2) * PW],
                        in0=th_b[:, lo * PW : (hi + 2) * PW],
                        in1=s_b[:, lo * PW : (hi + 2) * PW],
                        op=ALU.add,
                    ),
                    lambda: nc.vector.tensor_tensor(
                        out=v_b[:, lo * PW : (hi + 1) * PW],
                        in0=xpad[:, (base + lo) * PW : (base + hi + 1) * PW],
                        in1=xpad[:, (base + lo + 1) * PW : (base + hi + 2) * PW],
                        op=ALU.add,
                    ),
                    lambda: nc.vector.tensor_tensor(
                        out=tv_b[:, lo * PW : hi * PW],
                        in0=v_b[:, lo * PW : hi * PW],
                        in1=v_b[:, (lo + 1) * PW : (hi + 1) * PW],
                        op=ALU.add,
                    ),
                    lambda: nc.vector.tensor_tensor(
                        out=yx_b[:, lo * PW : hi * PW],
                        in0=tv_b[:, lo * PW + 2 : hi * PW + 2],
                        in1=tv_b[:, lo * PW : hi * PW],
                        op=ALU.subtract,
                    ),
                    lambda: ydy_eng.tensor_tensor(
                        out=yy_b[:, lo * PW : hi * PW],
                        in0=th_b[:, (lo + 2) * PW : (hi + 2) * PW],
                        in1=th_b[:, lo * PW : hi * PW],
                        op=ALU.subtract,
                    ),
                ]

            # prologue: band 0 in 4-row sub-bands so chunk-0 matmuls start early
            alloc_band(0)
            for sb in range(4):
                for op in conv_ops(0, 4 * sb, 4 * sb + 4):
                    op()
            # band b+1's ops are emitted only during band b's chunks —
            # emitting band b+2 early would head-block the in-order DVE
            # stream on a WAR dep (its buffers are still being read).
            pend = {}
            for b in range(1, 4):
                alloc_band(b)
                pend[b] = conv_ops(b, 0, 16)

            def emit_prepool():
                t1 = pp.tile([128, 524], F32, tag="pp_t1")
                vm = pp.tile([128, 524], F32, tag="pp_vm")
                t2 = pp.tile([128, 524], F32, tag="pp_t2")
                nc.vector.tensor_tensor(
                    out=t1[:, 0:520], in0=astrip[:, 0:520],
                    in1=astrip[:, 130:650], op=ALU.max,
                )
                nc.vector.tensor_tensor(
                    out=vm[:, 0:520], in0=t1[:, 0:520],
                    in1=astrip[:, 260:780], op=ALU.max,
                )
                nc.vector.tensor_tensor(
                    out=t2[:, 0:519], in0=vm[:, 0:519], in1=vm[:, 1:520],
                    op=ALU.max,
                )
                vmr = vm[:, 0:520].rearrange("p (r w) -> p r w", w=130)
                t2r = t2[:, 0:520].rearrange("p (r w) -> p r w", w=130)
                ppr = prepool[:, :].rearrange("p (r w) -> p r w", w=128)
                nc.vector.tensor_tensor(
                    out=ppr[:, 0:4, :], in0=t2r[:, 0:4, 0:128],
                    in1=vmr[:, 0:4, 2:130], op=ALU.max,
                )

            # ---- main chunk loop ----
            A = psp.tile([128, 2048], F32, tag="psA")
            Bp = psp.tile([128, 2048], F32, tag="psB")
            hp = [A, Bp]
            xpr = xpad[:, 0:NPAD].rearrange("p (r w) -> p r w", w=PW)
            x2r = x2[:, :].rearrange("p (r w) -> p r w", w=PW)

            for k in range(NCHUNK):
                if k == 2:
                    emit_prepool()
                b = k // 4
                lr0 = 4 * k - 16 * b
                yx_r = band_tiles[b][4][:, :].rearrange("p (r w) -> p r w", w=PW)
                yy_r = band_tiles[b][5][:, :].rearrange("p (r w) -> p r w", w=PW)
                rhss = [
                    xpr[:, 1 + 4 * k : 5 + 4 * k, 2:130],
                    yx_r[:, lr0 : lr0 + 4, 1:129],
                    yy_r[:, lr0 : lr0 + 4, 1:129],
                ]
                Q = hp[k % 2]           # dxp accumulator (bank 0)
                # PE stream stays gapless: both fc0 phases back-to-back,
                # then fc1 (drain deps resolve while fc0-gg1 runs)
                for gg in range(2):
                    P = hp[(k + gg) % 2]
                    for fi, feat in enumerate(("id", "dx", "dy")):
                        for j in range(4):
                            nc.tensor.matmul(
                                P[:, 512 * j : 512 * j + 512],
                                w0t[(feat, gg)][32 * j : 32 * j + 32, :],
                                rhss[fi][32 * j : 32 * j + 32],
                                start=(fi == 0),
                                stop=(fi == 2),
                                tile_position=(32 * j, 0),
                            )
                # drain split ACT/DVE: one big op each (ACT pays ~293ns
                # fixed cost per ACTIVATE); asymmetric across gg to balance
                rhs_t = []
                SPLITS = (1024, 1536)   # gg0: ACT 2 banks; gg1: ACT 3 banks
                for gg in range(2):
                    P = hp[(k + gg) % 2]
                    sp = SPLITS[gg]
                    rh = pp.tile([128, 2048], BF16, tag=f"rh{gg}", bufs=2,
                                 name=f"rh{gg}_{k}")
                    nc.scalar.activation(
                        out=rh[:, 0:sp], in_=P[:, 0:sp], func=ACTF.Relu
                    )
                    nc.vector.tensor_scalar_max(
                        rh[:, sp:2048], P[:, sp:2048], 0.0
                    )
                    rhs_t.append(rh)
                # fc1: DVE-drained banks first (they land earlier)
                JORDER = ((2, 3, 0, 1), (3, 0, 1, 2))
                for gg in range(2):
                    for j in JORDER[gg]:
                        nc.tensor.matmul(
                            Q[32 * j : 32 * j + 32, 0:512],
                            w1t[gg][:, :],
                            rhs_t[gg][:, 512 * j : 512 * j + 512],
                            start=(gg == 0),
                            stop=(gg == 1),
                            tile_position=(0, 32 * j),
                        )
                nxt = pend.get(b + 1)
                for _ in range(4):
                    if nxt:
                        nxt.pop(0)()
                st = pp.tile([128, 512], BF16, tag="st", bufs=2)
                nc.vector.tensor_tensor(
                    out=st[:, :],
                    in0=Q[:, 0:512],
                    in1=u16[:, k * CN : (k + 1) * CN],
                    op=ALU.mult,
                )
                nc.vector.tensor_tensor(
                    out=x2r[:, 1 + 4 * k : 5 + 4 * k, 2:130],
                    in0=xpr[:, 1 + 4 * k : 5 + 4 * k, 2:130],
                    in1=st[:, :].rearrange("p (r w) -> p r w", w=W),
                    op=ALU.add,
                )

            cpx_exc = cp.__exit__(None, None, None)
            del cpx_exc

            # ---- alive masks ----
            x2ap = x2[:, :]
            a2ap = a2strip[:, :]
            # strip s row r <- x2 padded row 4s+r (cols 2..130); strips 0-15
            # from half 0 (partition chan-3 row 3), 16-31 from half 1 (row 19)
            for r in range(6):
                src = _mk_ap(
                    x2ap, 3 * NPAD + PW * r + 2,
                    [[16 * NPAD, 8], [4 * PW, 16], [1, 128]],
                )
                dst = _mk_ap(a2ap, 130 * r + 1, [[780, 128], [1, 128]])
                nc.gpsimd.dma_start(out=dst, in_=src)
            nc.gpsimd.dma_start(
                out=_mk_ap(a2ap, 15 * 780 + 5 * 130 + 1, [[32 * 780, 4], [1, 128]]),
                in_=_mk_ap(x2ap, 19 * NPAD + 1 * PW + 2, [[32 * NPAD, 4], [1, 128]]),
            )
            nc.gpsimd.dma_start(
                out=_mk_ap(a2ap, 16 * 780 + 1, [[32 * 780, 4], [1, 128]]),
                in_=_mk_ap(
                    x2ap, 3 * NPAD + 64 * PW + 2, [[32 * NPAD, 4], [1, 128]]
                ),
            )

            postpool = pp.tile([128, 512], BF16, tag="postpool")
            t1b = pp.tile([128, 524], BF16, tag="ap_t1")
            vmb = pp.tile([128, 524], BF16, tag="ap_vm")
            t2b = pp.tile([128, 524], BF16, tag="ap_t2")
            nc.vector.tensor_tensor(
                out=t1b[:, 0:520], in0=a2strip[:, 0:520],
                in1=a2strip[:, 130:650], op=ALU.max,
            )
            nc.vector.tensor_tensor(
                out=vmb[:, 0:520], in0=t1b[:, 0:520],
                in1=a2strip[:, 260:780], op=ALU.max,
            )
            nc.vector.tensor_tensor(
                out=t2b[:, 0:519], in0=vmb[:, 0:519], in1=vmb[:, 1:520],
                op=ALU.max,
            )
            vmr2 = vmb[:, 0:520].rearrange("p (r w) -> p r w", w=130)
            t2r2 = t2b[:, 0:520].rearrange("p (r w) -> p r w", w=130)
            ppr2 = postpool[:, :].rearrange("p (r w) -> p r w", w=128)
            nc.vector.tensor_tensor(
                out=ppr2[:, 0:4, :], in0=t2r2[:, 0:4, 0:128],
                in1=vmr2[:, 0:4, 2:130], op=ALU.max,
            )
            pmin = pp.tile([128, 512], BF16, tag="pmin")
            nc.vector.tensor_tensor(
                out=pmin[:, :], in0=prepool[:, :], in1=postpool[:, :], op=ALU.min
            )
            lifes = pp.tile([128, 512], BF16, tag="lifes")
            nc.vector.tensor_scalar(
                out=lifes[:, :], in0=pmin[:, :], scalar1=0.1, scalar2=None,
                op0=ALU.is_gt,
            )

            # ---- life broadcast (PE) + final mask multiply + store ----
            for k4 in range(4):
                lps = hp[k4 % 2]
                for tl in range(4):
                    t = 4 * k4 + tl
                    nc.tensor.matmul(
                        lps[:, 512 * tl : 512 * tl + 512],
                        selt[:, 128 * t : 128 * t + 128],
                        lifes[:, 0:512],
                        start=True,
                        stop=True,
                    )
                ot = pp.tile([128, 2048], BF16, tag="ot", bufs=2)
                nc.vector.tensor_tensor(
                    out=ot[:, :],
                    in0=x2r[:, 1 + 16 * k4 : 17 + 16 * k4, 2:130],
                    in1=lps[:, :],
                    op=ALU.mult,
                )
                nc.sync.dma_start(
                    out=out_d[:, 2048 * k4 : 2048 * (k4 + 1)], in_=ot[:, :]
                )

    _split_multiwaits(nc)
    return nc


def host_prep(x, w0, w1, rand_mask):
    bf = ml_dtypes.bfloat16
    xt = np.ascontiguousarray(x.transpose(0, 3, 1, 2))  # [B, C, H, W]

    xp = np.zeros((B, 2, C, PR, PW), np.float32)
    xp[:, 0, :, 1:66, 2:130] = xt[:, :, 0:65, :]
    xp[:, 1, :, 0:65, 2:130] = xt[:, :, 63:128, :]
    xp = xp.astype(bf).reshape(B, 2, C, NPAD)

    u = (rand_mask[..., 0] <= 0.5).astype(np.float32).reshape(B, 2, 64, W)
    u16 = np.ascontiguousarray(
        np.broadcast_to(u[:, :, None], (B, 2, C, 64, W))
    ).astype(bf).reshape(B, 2, C, NPIX)

    apad = np.zeros((B, H + 2, 130), np.float32)
    apad[:, 1:129, 1:129] = x[..., 3]
    idx = 4 * np.arange(32)[:, None] + np.arange(6)[None, :]
    astr = apad[:, idx, :].reshape(B, 32, 780)  # [B, strip, 6*130]

    W0id = w0[:, 0::3]
    W0dx = w0[:, 1::3] / 8.0
    W0dy = w0[:, 2::3] / 8.0
    w0_arrs = {}
    for feat, Wm in (("id", W0id), ("dx", W0dx), ("dy", W0dy)):
        blk = Wm.T.astype(bf)  # [16 c, 128 o]
        for gg in range(2):
            t = np.zeros((128, 128), bf)
            for j in range(4):
                t[32 * j + 16 * gg : 32 * j + 16 * gg + 16, :] = blk
            w0_arrs[(feat, gg)] = t
    w1_arrs = []
    for gg in range(2):
        t = np.zeros((128, 32), bf)
        t[:, 16 * gg : 16 * gg + 16] = w1.T.astype(bf)
        w1_arrs.append(t)

    sel = np.zeros((128, 2048), bf)
    for t in range(16):
        for p in range(128):
            g = p // 16
            sel[16 * g + t, 128 * t + p] = 1.0

    in_maps = []
    for ci in range(N_CORES):
        sl = slice(IMGS * ci, IMGS * (ci + 1))
        m = {
            "xpad": np.ascontiguousarray(xp[sl]).reshape(128, NPAD),
            "u16": np.ascontiguousarray(u16[sl]).reshape(128, NPIX),
            "astrip": np.ascontiguousarray(astr[sl]).reshape(128, 780),
            "sel": sel,
            "w10": w1_arrs[0],
            "w11": w1_arrs[1],
        }
        for (feat, gg), arr in w0_arrs.items():
            m[f"w0{feat}{gg}"] = arr
        in_maps.append(m)
    return in_maps


def host_post(results):
    out = np.empty((B, H, W, C), np.float32)
    for ci in range(N_CORES):
        o = results[ci]["out"].astype(np.float32).reshape(IMGS, 2, C, 64, W)
        out[IMGS * ci : IMGS * (ci + 1)] = o.transpose(0, 1, 3, 4, 2).reshape(
            IMGS, H, W, C
        )
    return out


_CACHE = {}


def kernel(x, w0, w1, rand_mask, _trace=False):
    x = np.asarray(x, np.float32)
    w0 = np.asarray(w0, np.float32)
    w1 = np.asarray(w1, np.float32)
    rand_mask = np.asarray(rand_mask, np.float32)

    if "nc" not in _CACHE:
        _CACHE["nc"] = build_program()
    nc = _CACHE["nc"]

    in_maps = host_prep(x, w0, w1, rand_mask)
    res = bass_utils.run_bass_kernel_spmd(
        nc, in_maps, core_ids=list(range(N_CORES)), trace=_trace
    )
    _CACHE["last_result"] = res
    return host_post(res.results)
